# revision 45
# baseline (speedup 1.0000x reference)
"""Trainium2 Bass kernel for nn_BertEncoder_81604378624298 (segment_reduce).

Computation (per example b, X := sum_l emb[b,l]):
  word[g,d] = sum_{s: tg[b,s]==g} X[s,d]        (scatter-sum via one-hot matmul)
  seg[k,d]  = sum_{s: c[b,s]==k} X[s,d]         (composed scatter, c = sg[tg[s]])
  sent[d]   = sum_s X[s,d] / S                  (ones*(1/S) column in the lhsT)
  mask[k]   = k > sg[b,S-1]                     (ids are sorted)

Design notes:
  - Scatter-sums run on the PE as one-hot matmuls.  The moving operand is
    split X = hi + lo (hi = bf16(X), lo = bf16(X - hi)); one-hot weights are
    exact in bf16 and PSUM accumulates in fp32, so the only error is the
    ~2^-17 relative rounding of the hi/lo decomposition.
  - The layer sum over L=4 happens on the DMA engines: per (example, s-tile)
    a 4-link chain — layer 0 is a plain HWDGE copy, layers 1..3 accumulate
    via SWDGE (accum_op=add).  All input chains are issued before any
    compute, layer-outer, so no chain ever stalls the queues head-of-line
    and output DMAs never delay a later example's inputs.
  - token_group_ids < 300 (spec fill_max), so word rows >= maxid+1 are
    identically zero; the runner pre-zeros outputs, so only ceil((maxid+1)/128)
    word tiles are computed.  The LAST (partial) word tile has spare lhsT
    columns, so the seg one-hot (24 cols, composed ids c = sg[tg[s]]) and the
    sent 1/S column are appended to it — the segment/sentence outputs fall
    out of the same matmul pass and there is no separate stage 2.
  - All id columns ship as ONE host-prepared fp32 constant tensor (a single
    DMA instead of dozens of tiny ones); the iota compare operand is
    generated on-chip by gpsimd so no replicated data crosses HBM.

Sharding: data-parallel over batch, B=32 across 8 cores -> 4 examples/core.
Same program on every core, per-core input shards, no collectives.
"""

from contextlib import ExitStack

import numpy as np

import concourse.bacc as bacc
import concourse.bass as bass
import concourse.mybir as mybir
import concourse.tile as tile
from concourse.bass_utils import run_bass_kernel_spmd

B, L, S, D = 32, 4, 512, 768
NSEG = 24
NCORES = 8
EX = B // NCORES          # examples per core
P = 128                   # partitions
NT = S // P               # s-tiles per example

f32 = mybir.dt.float32
bf16 = mybir.dt.bfloat16
u8 = mybir.dt.uint8
AL = mybir.AluOpType

_CHUNKS = ((0, 512), (512, D))  # matmul moving-dim chunks (<=512 fp32 psum per bank)

# consts tensor column layout: [tg cols | composed cols | last] (iota is
# generated on-chip; only genuinely per-partition id data ships from host)
_CTG = 0                    # tg id columns start
_CCM = EX * NT              # composed id columns start
_CLS = 2 * EX * NT          # per-example last-segment-id columns start
_CW = 2 * EX * NT + EX      # total width


def _build_program(n_full: int, tail_cols: int) -> bass.Bass:
    """n_full: number of full 128-column word tiles; tail_cols: word columns
    in the final partial tile (tail_cols + NSEG + 1 <= 128; may be 0).
    The final tile's lhsT is [oh(tail) | seg one-hot | 1/S]."""
    assert 0 <= tail_cols and tail_cols + NSEG + 1 <= P
    mg0 = n_full * P                 # first word row of the merged tail tile
    mgw = tail_cols + NSEG + 1       # merged lhsT width

    nc = bacc.Bacc("TRN2", target_bir_lowering=False, debug=False)
    emb_d = nc.dram_tensor("emb", [EX, L, S, D], f32, kind="ExternalInput").ap()
    consts_d = nc.dram_tensor("consts", [P, _CW], f32, kind="ExternalInput").ap()
    word_d = nc.dram_tensor("word", [EX, S, D], f32, kind="ExternalOutput").ap()
    sent_d = nc.dram_tensor("sent", [EX, D], f32, kind="ExternalOutput").ap()
    seg_d = nc.dram_tensor("seg", [EX, NSEG, D], f32, kind="ExternalOutput").ap()
    mask_d = nc.dram_tensor("mask", [EX, NSEG], u8, kind="ExternalOutput").ap()

    with tile.TileContext(nc) as tc, ExitStack() as ctx:
        cpool = ctx.enter_context(tc.tile_pool(name="const", bufs=1))
        xpool = ctx.enter_context(tc.tile_pool(name="x", bufs=EX * NT))
        xhp = ctx.enter_context(tc.tile_pool(name="xh", bufs=NT + 4))
        xlp = ctx.enter_context(tc.tile_pool(name="xl", bufs=NT + 4))
        ohp = ctx.enter_context(tc.tile_pool(name="oh", bufs=EX * NT))
        sgp = ctx.enter_context(tc.tile_pool(name="sg1h", bufs=EX * NT))
        wpool = ctx.enter_context(tc.tile_pool(name="w", bufs=NT + 4))
        mrow = ctx.enter_context(tc.tile_pool(name="mrow", bufs=EX))
        # t-outer keeps all n_full+1 PSUM groups of an example open at once;
        # that fits the 8 banks only for n_full <= 3.  For n_full == 4 (full
        # id range) fall back to g-outer accumulation.
        t_outer = n_full <= 3
        wps = ctx.enter_context(tc.tile_pool(
            name="wps", bufs=(1 if t_outer else 2), space="PSUM"))
        sps = ctx.enter_context(tc.tile_pool(
            name="sps", bufs=(2 if n_full <= 2 else 1), space="PSUM"))

        consts_t = cpool.tile([P, _CW], f32)
        nc.sync.dma_start(consts_t[:], consts_d[:, :])
        # iota is generated on-chip (saves a 256KB replicated-row DMA); the
        # int32 values are exact under the fp32 ALU cast of is_equal/is_gt
        iota_t = cpool.tile([P, S], mybir.dt.int32)
        nc.gpsimd.iota(iota_t[:], [[1, S]], channel_multiplier=0)
        iota_v = iota_t[:, 0:S]
        # absorb the consts-DMA and iota waits into DVE program order so
        # later TensorScalarPtr ops only ever need one extra sync-wait
        probe_t = cpool.tile([1, 1], f32)
        nc.vector.tensor_copy(probe_t[:], consts_t[0:1, 0:1])
        probe2_t = cpool.tile([1, 1], f32)
        nc.vector.tensor_copy(probe2_t[:], iota_t[0:1, 0:1])

        # Phase A: all input loads.  Two 2-link chains per tile:
        # x_a = l0 (HWDGE) + l1 (SWDGE accum), x_b = l2 + l3; SWDGE links are
        # mutually independent so the Pool queue never waits.
        xas, xbs = [], []
        for b in range(EX):
            rowa, rowb = [], []
            for t in range(NT):
                x_t = xpool.tile([P, D], f32)
                nc.sync.dma_start(x_t[:], emb_d[b, 0, t * P:(t + 1) * P, :])
                rowa.append(x_t)
                xb_t = xpool.tile([P, D], f32, tag="xb")
                nc.sync.dma_start(xb_t[:], emb_d[b, 2, t * P:(t + 1) * P, :])
                rowb.append(xb_t)
            xas.append(rowa)
            xbs.append(rowb)
        for b in range(EX):
            for t in range(NT):
                for layer, rows in ((1, xas), (3, xbs)):
                    nc.gpsimd.dma_start(
                        rows[b][t][:], emb_d[b, layer, t * P:(t + 1) * P, :],
                        accum_op=AL.add,
                    )

        # Phase A2: everything that depends only on consts — one-hot builds
        # and masks for ALL examples — runs up front, off the critical path.
        all_oh, all_mg = [], []
        for b in range(EX):
            ohs, sgs = [], []
            for t in range(NT):
                j = b * NT + t
                tg_col = consts_t[:, _CTG + j:_CTG + j + 1]
                cm_col = consts_t[:, _CCM + j:_CCM + j + 1]
                if n_full > 0:
                    oh_t = ohp.tile([P, n_full * P], bf16)
                    nc.vector.tensor_scalar(
                        oh_t[:], iota_v[:, 0:n_full * P], tg_col, None, AL.is_equal
                    )
                    ohs.append(oh_t)
                mg_t = sgp.tile([P, mgw], bf16)
                if tail_cols > 0:
                    nc.vector.tensor_scalar(
                        mg_t[:, 0:tail_cols], iota_v[:, mg0:mg0 + tail_cols],
                        tg_col, None, AL.is_equal,
                    )
                nc.vector.tensor_scalar(
                    mg_t[:, tail_cols:tail_cols + NSEG], iota_v[:, 0:NSEG],
                    cm_col, None, AL.is_equal,
                )
                nc.vector.memset(mg_t[:, mgw - 1:mgw], 1.0 / S)
                sgs.append(mg_t)
            all_oh.append(ohs)
            all_mg.append(sgs)
            m_t = mrow.tile([1, NSEG], u8)
            nc.vector.tensor_scalar(
                m_t[:], iota_v[0:1, 0:NSEG], consts_t[0:1, _CLS + b:_CLS + b + 1],
                None, AL.is_gt,
            )
            nc.sync.dma_start(mask_d[b:b + 1, :], m_t[:])

        # Phase B: per-example combine + hi/lo splits and matmuls.  Matmuls
        # are emitted t-outer so every PSUM group accumulates incrementally
        # as its s-tiles arrive; after the LAST tile lands only one
        # accumulation step per group remains.
        for b in range(EX):
            ohs, sgs = all_oh[b], all_mg[b]
            xhs, xls = [], []
            for t in range(NT):
                x_t = xas[b][t]
                nc.vector.tensor_tensor(x_t[:], x_t[:], xbs[b][t][:], AL.add)
                xh_t = xhp.tile([P, D], bf16)
                nc.vector.tensor_copy(xh_t[:], x_t[:])
                xl_t = xlp.tile([P, D], bf16)
                nc.vector.tensor_tensor(xl_t[:], x_t[:], xh_t[:], AL.subtract)
                xhs.append(xh_t)
                xls.append(xl_t)

            if t_outer:
                wp_ts = []
                for g in range(n_full):
                    wp_g = wps.tile([P, D], f32, tag=f"wp{g}")
                    wp_ts.append(wp_g)
                sp_t = sps.tile([mgw, D], f32)
                for t in range(NT):
                    for g in range(n_full):
                        gsl = slice(g * P, (g + 1) * P)
                        for c0, c1 in _CHUNKS:
                            nc.tensor.matmul(
                                wp_ts[g][:, c0:c1], lhsT=ohs[t][:, gsl],
                                rhs=xhs[t][:, c0:c1], start=(t == 0), stop=False,
                            )
                            nc.tensor.matmul(
                                wp_ts[g][:, c0:c1], lhsT=ohs[t][:, gsl],
                                rhs=xls[t][:, c0:c1], start=False, stop=(t == NT - 1),
                            )
                    for c0, c1 in _CHUNKS:
                        nc.tensor.matmul(
                            sp_t[:, c0:c1], lhsT=sgs[t][:],
                            rhs=xhs[t][:, c0:c1], start=(t == 0), stop=False,
                        )
                        nc.tensor.matmul(
                            sp_t[:, c0:c1], lhsT=sgs[t][:],
                            rhs=xls[t][:, c0:c1], start=False, stop=(t == NT - 1),
                        )
            else:
                wp_ts = []
                for g in range(n_full):
                    wp_g = wps.tile([P, D], f32, tag="wp")
                    wp_ts.append(wp_g)
                    gsl = slice(g * P, (g + 1) * P)
                    for c0, c1 in _CHUNKS:
                        for t in range(NT):
                            nc.tensor.matmul(
                                wp_g[:, c0:c1], lhsT=ohs[t][:, gsl],
                                rhs=xhs[t][:, c0:c1], start=(t == 0), stop=False,
                            )
                            nc.tensor.matmul(
                                wp_g[:, c0:c1], lhsT=ohs[t][:, gsl],
                                rhs=xls[t][:, c0:c1], start=False, stop=(t == NT - 1),
                            )
                sp_t = sps.tile([mgw, D], f32)
                for c0, c1 in _CHUNKS:
                    for t in range(NT):
                        nc.tensor.matmul(
                            sp_t[:, c0:c1], lhsT=sgs[t][:],
                            rhs=xhs[t][:, c0:c1], start=(t == 0), stop=False,
                        )
                        nc.tensor.matmul(
                            sp_t[:, c0:c1], lhsT=sgs[t][:],
                            rhs=xls[t][:, c0:c1], start=False, stop=(t == NT - 1),
                        )

            for g in range(n_full):
                w_t = wpool.tile([P, D], f32)
                # on the final example DVE is idle: split copies across
                # engines so the tail output DMAs are not copy-serialized
                if b == EX - 1 and g % 2 == 0:
                    nc.vector.tensor_copy(w_t[:], wp_ts[g][:])
                else:
                    nc.scalar.copy(w_t[:], wp_ts[g][:])
                nc.sync.dma_start(word_d[b, g * P:(g + 1) * P, :], w_t[:])
            sq_t = wpool.tile([mgw, D], f32, tag="sq")
            nc.scalar.copy(sq_t[:], sp_t[:])
            if tail_cols > 0:
                nc.sync.dma_start(
                    word_d[b, mg0:mg0 + tail_cols, :], sq_t[0:tail_cols, :])
            nc.sync.dma_start(seg_d[b, :, :], sq_t[tail_cols:tail_cols + NSEG, :])
            nc.sync.dma_start(sent_d[b:b + 1, :], sq_t[mgw - 1:mgw, :])
    nc.compile()
    return nc


_NC_CACHE = {}


def _get_program(n_full: int = 2, tail_cols: int = 44) -> bass.Bass:
    key = (n_full, tail_cols)
    if key not in _NC_CACHE:
        _NC_CACHE[key] = _build_program(n_full, tail_cols)
    return _NC_CACHE[key]


def _plan(max_id: int):
    """Pick (n_full, tail_cols) so that tail_cols + NSEG + 1 <= 128."""
    nwords = max_id + 1
    n_tiles = -(-nwords // P)                 # tiles needed to cover all words
    tail_cols = nwords - (n_tiles - 1) * P    # 1..128
    if tail_cols + NSEG + 1 <= P:
        return n_tiles - 1, tail_cols         # merge the partial tile
    return n_tiles, 0                         # tail tile carries seg+sent only


def _prepare_in_maps(embeddings, token_group_ids, segment_ids):
    emb = np.ascontiguousarray(embeddings, dtype=np.float32)
    tg = np.asarray(token_group_ids).astype(np.int64)
    sg = np.asarray(segment_ids).astype(np.int64)
    comp = np.take_along_axis(sg, tg, axis=1)          # c[b,s] = sg[b, tg[b,s]]
    last = sg[:, S - 1]                                # max segment id (sorted)

    in_maps = []
    for c in range(NCORES):
        bs = slice(c * EX, (c + 1) * EX)
        consts = np.empty((P, _CW), dtype=np.float32)
        # id columns: col j=b*NT+t holds ids[b, t*128 + p] on partition p
        consts[:, _CTG:_CCM] = (
            tg[bs].reshape(EX * NT, P).T.astype(np.float32))
        consts[:, _CCM:_CLS] = (
            comp[bs].reshape(EX * NT, P).T.astype(np.float32))
        consts[:, _CLS:_CW] = last[bs][None, :].astype(np.float32)
        in_maps.append({"emb": emb[bs], "consts": consts})
    return in_maps


def _run(in_maps, n_full: int = 2, tail_cols: int = 44, **kwargs):
    return run_bass_kernel_spmd(
        _get_program(n_full, tail_cols), in_maps,
        core_ids=list(range(NCORES)), **kwargs
    )


def _assemble(results):
    word = np.concatenate([r["word"] for r in results], axis=0)
    sent = np.concatenate([r["sent"] for r in results], axis=0)
    seg = np.concatenate([r["seg"] for r in results], axis=0)
    mask = np.concatenate([r["mask"] for r in results], axis=0).astype(np.bool_)
    return word, sent, seg, mask


def kernel(embeddings, token_group_ids, segment_ids):
    token_group_ids = np.asarray(token_group_ids)
    in_maps = _prepare_in_maps(
        np.asarray(embeddings), token_group_ids, np.asarray(segment_ids)
    )
    n_full, tail_cols = _plan(int(token_group_ids.max()))
    res = _run(in_maps, n_full=n_full, tail_cols=tail_cols)
    return _assemble(res.results)
